# revision 15
# baseline (speedup 1.0000x reference)
"""Multi-head causal attention (B=4, S=2048, D=1024, H=16) on 8 TRN2 cores.

Sharding: data-parallel over batch (4) x tensor-parallel over heads (2 groups
of 8 heads). Core c handles batch c//2, head-group c%2. Each core computes
q/k/v projections for its 8 heads, causal flash-style attention, and a partial
output projection against its row-shard of Wp. Host sums the two partials per
batch and adds the bias terms (bp + bv @ Wp, which commute with the row-sum).

v2 pipeline notes:
- Host passes x pre-transposed ([D, S]) and all big operands in bf16: no
  on-chip casts, no PE transposes, and half the startup DMA bytes.
- Causal masking via gpsimd.affine_select (triangle strips) + DVE memsets for
  the fully-masked column ranges; no mask tensor at all. This keeps the mask
  work off the Vector FIFO, which previously head-of-line-blocked the AV
  matmuls behind the normalize chain and let the PE HAM re-throttle.
- The softmax-denominator reciprocal still round-trips through DRAM to spread
  512 lanes, but is split into stages (A: evacuate+gather DMAs, B: reciprocal+
  scatter DMAs, C: normalize multiply) that pop from the shared pipeline queue
  one-per-S-matmul, so each stage's DMA completes before its consumer issues.
"""

import numpy as np

B, S, D, H = 4, 2048, 1024, 16
HD = D // H            # 64
HPC = 8                # heads per core
LCOL = HPC * HD        # 512 local columns
NSG = 4                # seq groups of 512
SG = S // NSG          # 512
NKB = S // 128         # 16 key blocks of 128

_CACHE = {}


def _build(cdt_name="bfloat16", debug_dump=False):
    import concourse.bass as bass
    import concourse.tile as tile
    from concourse import bacc, mybir

    f32 = mybir.dt.float32
    cdt = getattr(mybir.dt, cdt_name)

    nc = bacc.Bacc("TRN2", target_bir_lowering=False, debug=False)

    xt_d = nc.dram_tensor("xt", [NSG, D, SG], cdt, kind="ExternalInput")
    wq_d = nc.dram_tensor("wq", [D, LCOL], cdt, kind="ExternalInput")
    wk_d = nc.dram_tensor("wk", [D, LCOL], cdt, kind="ExternalInput")
    wv_d = nc.dram_tensor("wv", [D, LCOL], cdt, kind="ExternalInput")
    bq_d = nc.dram_tensor("bq", [LCOL], f32, kind="ExternalInput")
    bk_d = nc.dram_tensor("bk", [LCOL], f32, kind="ExternalInput")
    wp_d = nc.dram_tensor("wp", [LCOL, D], cdt, kind="ExternalInput")
    out_d = nc.dram_tensor("out", [S, D], cdt, kind="ExternalOutput")

    Exp = mybir.ActivationFunctionType.Exp
    GE = mybir.AluOpType.is_ge

    with tile.TileContext(nc) as tc:
        with (
            tc.tile_pool(name="consts", bufs=1) as consts,
            tc.tile_pool(name="acts", bufs=1) as acts,
            tc.tile_pool(name="pp", bufs=6) as pp,
            tc.tile_pool(name="recp", bufs=2) as recp,
            tc.tile_pool(name="orp", bufs=3) as orp,
            tc.tile_pool(name="rp", bufs=2) as rp,
            tc.tile_pool(name="outp", bufs=4) as outp,
            tc.tile_pool(name="ps_a", bufs=2, space="PSUM") as ps_a,
            tc.tile_pool(name="ps_s", bufs=2, space="PSUM") as ps_s,
            tc.tile_pool(name="ps_o", bufs=2, space="PSUM") as ps_o,
        ):
            zero_g = nc.gpsimd.to_reg(0.0)

            # ---- constants / weights (bf16 direct, no staging casts) ------
            bq_sb = consts.tile([128, 4], f32)
            bk_sb = consts.tile([128, 4], f32)

            # persistent activations
            xT = acts.tile([128, 8, S], cdt)       # [d%128, dblk, seq]
            qT = acts.tile([128, 4, S], cdt)       # [col%128, colblk, seq]
            kT = acts.tile([128, 4, S], cdt)
            v_ext = acts.tile([128, NKB, HPC * (HD + 1)], cdt)  # per kb: 8*(64+1)
            oT = acts.tile([128, 4, S], cdt)
            for l in range(HPC):                   # ones columns for denominator
                nc.vector.memset(v_ext[:, :, 65 * l + 64: 65 * l + 65], 1.0)

            wq_sb = consts.tile([128, 8, LCOL], cdt)
            wk_sb = consts.tile([128, 8, LCOL], cdt)
            wv_sb = consts.tile([128, 8, LCOL], cdt)
            wp_sb = consts.tile([128, 4, D], cdt)

            def load_weights_early():
                # xt(g0) is already queued ahead of these; wp (only needed by
                # the g0 projection, which runs during g1) loads last, and the
                # tiny biases load after the weights they are added to.
                nc.sync.dma_start(out=wq_sb, in_=wq_d.ap()
                                  .rearrange("(c p) n -> p c n", p=128))
                nc.sync.dma_start(out=wk_sb, in_=wk_d.ap()
                                  .rearrange("(c p) n -> p c n", p=128))
                nc.sync.dma_start(out=bq_sb,
                                  in_=bq_d.ap().rearrange("(c p) -> p c", p=128))
                nc.sync.dma_start(out=bk_sb,
                                  in_=bk_d.ap().rearrange("(c p) -> p c", p=128))
                nc.sync.dma_start(out=wv_sb, in_=wv_d.ap()
                                  .rearrange("(c p) n -> p c n", p=128))

            def phase1_gen(g):
                """qk/v for seq group g; yields between chunks for interleave."""
                nc.sync.dma_start(
                    out=xT[:, :, g * SG:(g + 1) * SG],
                    in_=xt_d.ap()[g].rearrange("(c p) s -> p c s", p=128))
                yield
                for w_sb, b_sb, dstT in ((wq_sb, bq_sb, qT), (wk_sb, bk_sb, kT)):
                    for m in range(4):
                        pq = ps_a.tile([128, SG], f32, name="pq", tag="ps_a")
                        for dc in range(8):
                            nc.tensor.matmul(
                                pq, lhsT=w_sb[:, dc, 128 * m:128 * (m + 1)],
                                rhs=xT[:, dc, g * SG:(g + 1) * SG],
                                start=(dc == 0), stop=(dc == 7))
                        nc.vector.tensor_scalar_add(
                            dstT[:, m, g * SG:(g + 1) * SG], pq,
                            b_sb[:, m:m + 1])
                        yield
                for s in range(4):
                    pv = ps_a.tile([128, LCOL], f32, name="pv", tag="ps_a")
                    for dc in range(8):
                        nc.tensor.matmul(
                            pv, lhsT=xT[:, dc, g * SG + 128 * s:g * SG + 128 * (s + 1)],
                            rhs=wv_sb[:, dc, :], start=(dc == 0), stop=(dc == 7))
                    kb = 4 * g + s
                    nc.vector.tensor_copy(
                        v_ext[:, kb, :].rearrange("p (h e) -> p h e", e=65)[:, :, 0:64],
                        pv.rearrange("p (h e) -> p h e", e=64))
                    yield

            def attn(hp, qg, tick=lambda: None, queue=None):
                """One (head-pair, q-group) attention unit for heads (2hp,
                2hp+1). The two heads' K=64 score matmuls target different
                64-row PE tiles (partitions 0-63 / 64-127) and run
                concurrently; one exp / affine_select instruction covers both
                heads' blocks for the same key block. exp/AV emission lags S
                emission via `queue` (shared across units); the normalize
                chain is staged so no engine FIFO ever waits on a slow hop."""
                nkb = 4 * qg + 4
                q_sl = slice(qg * SG, (qg + 1) * SG)
                psum_os = [ps_o.tile([65, SG], f32, name=f"psum_o{j}",
                                     tag="ps_o") for j in range(2)]
                st = {}

                def exp_av(kb, psum_s):
                    p_sb = pp.tile([128, 2, SG], cdt, name="p_sb", tag="p_sb")
                    qoff = 128 * kb - 512 * qg
                    if qoff > 0:
                        nc.scalar.activation(p_sb[:, :, qoff:SG],
                                             psum_s[:, :, qoff:SG], Exp,
                                             scale=0.125)
                        nc.vector.memset(p_sb[:, :, 0:qoff], 0.0)
                    else:
                        nc.scalar.activation(p_sb, psum_s, Exp, scale=0.125)
                    if kb >= 4 * qg:  # diagonal block: triangle strip, both j
                        q0 = max(qoff, 0)
                        w = min(128, SG - q0)
                        nc.gpsimd.affine_select(
                            out=p_sb[:, :, q0:q0 + w],
                            in_=p_sb[:, :, q0:q0 + w],
                            compare_op=GE, fill=zero_g,
                            base=512 * qg + q0 - 128 * kb,
                            channel_multiplier=-1, pattern=[[0, 2], [1, w]])
                    for j in range(2):
                        h = 2 * hp + j
                        nc.tensor.matmul(
                            psum_os[j], lhsT=v_ext[:, kb, 65 * h:65 * h + 65],
                            rhs=p_sb[:, j, :], start=(kb == 0),
                            stop=(kb == nkb - 1))

                def stage_a(_a, _b):
                    for j in range(2):
                        o_raw = orp.tile([65, SG], f32, name=f"o_raw{j}",
                                         tag="o_raw")
                        nc.vector.tensor_copy(o_raw, psum_os[j])
                        # den row to physical partition 0 (custom DVE/gpsimd
                        # ops misbehave at non-zero base partitions)
                        den0 = recp.tile([1, SG], f32, name=f"den0{j}",
                                         tag="den0")
                        nc.sync.dma_start(out=den0, in_=o_raw[64:65, :])
                        st[("o_raw", j)], st[("den0", j)] = o_raw, den0

                def stage_b(_a, _b):
                    for j in range(2):
                        rec = recp.tile([1, SG], f32, name=f"rec{j}",
                                        tag="rec")
                        nc.vector.reciprocal_approx_fast(rec, st[("den0", j)])
                        r_sb = rp.tile([64, SG], f32, name=f"r_sb{j}",
                                       tag="r_sb")
                        nc.gpsimd.partition_broadcast(r_sb, rec, channels=64)
                        st[("r_sb", j)] = r_sb

                def stage_c(_a, _b):
                    for j in range(2):
                        po_sl = slice(64 * j, 64 * j + 64)
                        nc.vector.tensor_mul(oT[po_sl, hp, q_sl],
                                             st[("o_raw", j)][0:64, :],
                                             st[("r_sb", j)])

                # Two key blocks of S before their AVs: the 64-row S tiles
                # and full-array AV matmuls force a PE mode switch at each
                # transition, so batch to halve the switch count.
                for kb2 in range(0, nkb, 2):
                    pss = []
                    for kb in (kb2, kb2 + 1):
                        psum_s = ps_s.tile([128, 2, SG], f32, name="psum_s",
                                           tag="ps_s")
                        for j in range(2):
                            po_sl = slice(64 * j, 64 * j + 64)
                            nc.tensor.matmul(
                                psum_s[:, j, :],
                                lhsT=kT[po_sl, hp, 128 * kb:128 * (kb + 1)],
                                rhs=qT[po_sl, hp, q_sl], start=True, stop=True)
                            tick()
                        pss.append((kb, psum_s))
                    for kb, ps in pss:
                        queue.append((exp_av, kb, ps))
                        if len(queue) > 2:
                            fn, a, b = queue.pop(0)
                            fn(a, b)
                queue.append((stage_a, None, None))
                queue.append((stage_b, None, None))
                queue.append((stage_c, None, None))
                while len(queue) > 4:
                    fn, a, b = queue.pop(0)
                    fn(a, b)

            def proj_gen(g):
                for s in range(4):
                    sb = 4 * g + s
                    for j in range(2):
                        ppr = ps_a.tile([128, SG], f32, name="ppr", tag="ps_a")
                        for c in range(4):
                            nc.tensor.matmul(
                                ppr, lhsT=oT[:, c, 128 * sb:128 * (sb + 1)],
                                rhs=wp_sb[:, c, j * SG:(j + 1) * SG],
                                start=(c == 0), stop=(c == 3))
                        o_sb = outp.tile([128, SG], cdt, name="o_sb", tag="o_sb")
                        nc.vector.tensor_copy(o_sb, ppr)
                        nc.sync.dma_start(
                            out=out_d.ap()[128 * sb:128 * (sb + 1),
                                           j * SG:(j + 1) * SG],
                            in_=o_sb)
                        yield

            # Emission interleaves last-group projections and next-group qkv
            # into the exp-bound attention stretches so the PE stays fed.
            p1_0 = phase1_gen(0)
            next(p1_0)            # queue the xt(g0) DMA first...
            load_weights_early()  # ...then the weights it computes against
            for _ in p1_0:
                pass
            nc.sync.dma_start(out=wp_sb, in_=wp_d.ap()
                              .rearrange("(c p) n -> p c n", p=128))
            for g in range(NSG):
                fill = []
                if g > 0:
                    fill.append(proj_gen(g - 1))
                if g < NSG - 1:
                    fill.append(phase1_gen(g + 1))
                n_chunks = (8 if g > 0 else 0) + (13 if g < NSG - 1 else 0)
                n_ticks = HPC * (4 * g + 4)
                stride = max(1, n_ticks // max(n_chunks, 1))
                state = {"i": 0}

                def feed():
                    if fill:
                        try:
                            next(fill[0])
                        except StopIteration:
                            fill.pop(0)

                def tick():
                    state["i"] += 1
                    if state["i"] % stride == 0:
                        feed()

                queue = []
                for hp in range(HPC // 2):
                    attn(hp, g, tick, queue)
                while queue:  # group boundary: drain before proj fill reads oT
                    fn, a, b = queue.pop(0)
                    fn(a, b)
                    feed()  # keep the PE fed during the drain
                for gen in fill:  # drain any remaining chunks
                    for _ in gen:
                        pass
            for _ in proj_gen(NSG - 1):
                pass

            if debug_dump:
                for nm, t in (("qT", qT), ("kT", kT), ("v_ext", v_ext),
                              ("oT", oT)):
                    dmp = nc.dram_tensor(f"dump_{nm}", list(t.shape), cdt,
                                         kind="ExternalOutput")
                    nc.sync.dma_start(out=dmp.ap(), in_=t)

    nc.compile()
    return nc


def _get_nc():
    if "nc" not in _CACHE:
        _CACHE["nc"] = _build()
    return _CACHE["nc"]


def make_in_maps(x, Wq, bq, Wk, bk, Wv, Wp):
    import ml_dtypes
    bf16 = ml_dtypes.bfloat16
    in_maps = []
    for c in range(8):
        b, hg = c // 2, c % 2
        hs = slice(hg * HPC, (hg + 1) * HPC)
        in_maps.append({
            "xt": np.ascontiguousarray(
                x[b].T.reshape(D, NSG, SG).transpose(1, 0, 2).astype(bf16)),
            "wq": np.ascontiguousarray(
                Wq[hs].transpose(1, 0, 2).reshape(D, LCOL).astype(bf16)),
            "wk": np.ascontiguousarray(
                Wk[hs].transpose(1, 0, 2).reshape(D, LCOL).astype(bf16)),
            "wv": np.ascontiguousarray(
                Wv[hs].transpose(1, 0, 2).reshape(D, LCOL).astype(bf16)),
            "bq": np.ascontiguousarray(bq[hs].reshape(LCOL).astype(np.float32)),
            "bk": np.ascontiguousarray(bk[hs].reshape(LCOL).astype(np.float32)),
            "wp": np.ascontiguousarray(
                Wp[hg * LCOL:(hg + 1) * LCOL, :].astype(bf16)),
        })
    return in_maps


def combine(results, Wp, bv, bp):
    """Unshard: sum the two head-group partials per batch + linear bias terms."""
    add = bp + bv.reshape(D) @ Wp
    out = np.empty((B, S, D), np.float32)
    for b in range(B):
        out[b] = (results[2 * b]["out"].astype(np.float32)
                  + results[2 * b + 1]["out"].astype(np.float32) + add)
    return out


def kernel(x, Wq, bq, Wk, bk, Wv, bv, Wp, bp):
    from concourse.bass_utils import run_bass_kernel_spmd

    x = np.asarray(x, np.float32)
    Wq = np.asarray(Wq, np.float32)
    Wk = np.asarray(Wk, np.float32)
    Wv = np.asarray(Wv, np.float32)
    bq = np.asarray(bq, np.float32)
    bk = np.asarray(bk, np.float32)
    bv = np.asarray(bv, np.float32)
    Wp = np.asarray(Wp, np.float32)
    bp = np.asarray(bp, np.float32)

    nc = _get_nc()
    in_maps = make_in_maps(x, Wq, bq, Wk, bk, Wv, Wp)
    res = run_bass_kernel_spmd(nc, in_maps, core_ids=list(range(8)))
    return combine(res.results, Wp, bv, bp)


# revision 19
# speedup vs baseline: 1.0006x; 1.0006x over previous
"""Multi-head causal attention (B=4, S=2048, D=1024, H=16) on 8 TRN2 cores.

Sharding: data-parallel over batch (4) x tensor-parallel over heads (2 groups
of 8 heads). Core c handles batch c//2, head-group c%2. Each core computes
q/k/v projections for its 8 heads, causal flash-style attention, and a partial
output projection against its row-shard of Wp. Host sums the two partials per
batch and adds the bias terms (bp + bv @ Wp, which commute with the row-sum).

v2 pipeline notes:
- Host passes x pre-transposed ([D, S]) and all big operands in bf16: no
  on-chip casts, no PE transposes, and half the startup DMA bytes.
- Causal masking via gpsimd.affine_select (triangle strips) + DVE memsets for
  the fully-masked column ranges; no mask tensor at all. This keeps the mask
  work off the Vector FIFO, which previously head-of-line-blocked the AV
  matmuls behind the normalize chain and let the PE HAM re-throttle.
- The softmax-denominator reciprocal still round-trips through DRAM to spread
  512 lanes, but is split into stages (A: evacuate+gather DMAs, B: reciprocal+
  scatter DMAs, C: normalize multiply) that pop from the shared pipeline queue
  one-per-S-matmul, so each stage's DMA completes before its consumer issues.
"""

import numpy as np

B, S, D, H = 4, 2048, 1024, 16
HD = D // H            # 64
HPC = 8                # heads per core
LCOL = HPC * HD        # 512 local columns
NSG = 4                # seq groups of 512
SG = S // NSG          # 512
NKB = S // 128         # 16 key blocks of 128

_CACHE = {}


def _build(cdt_name="bfloat16", debug_dump=False):
    import concourse.bass as bass
    import concourse.tile as tile
    from concourse import bacc, mybir

    f32 = mybir.dt.float32
    cdt = getattr(mybir.dt, cdt_name)

    nc = bacc.Bacc("TRN2", target_bir_lowering=False, debug=False)

    xt_d = nc.dram_tensor("xt", [NSG, D, SG], cdt, kind="ExternalInput")
    wq_d = nc.dram_tensor("wq", [D, LCOL], cdt, kind="ExternalInput")
    wk_d = nc.dram_tensor("wk", [D, LCOL], cdt, kind="ExternalInput")
    wv_d = nc.dram_tensor("wv", [D, LCOL], cdt, kind="ExternalInput")
    bq_d = nc.dram_tensor("bq", [LCOL], f32, kind="ExternalInput")
    bk_d = nc.dram_tensor("bk", [LCOL], f32, kind="ExternalInput")
    wp_d = nc.dram_tensor("wp", [LCOL, D], cdt, kind="ExternalInput")
    out_d = nc.dram_tensor("out", [S, D], cdt, kind="ExternalOutput")

    Exp = mybir.ActivationFunctionType.Exp
    GE = mybir.AluOpType.is_ge

    with tile.TileContext(nc) as tc:
        with (
            tc.tile_pool(name="consts", bufs=1) as consts,
            tc.tile_pool(name="acts", bufs=1) as acts,
            tc.tile_pool(name="pp", bufs=6) as pp,
            tc.tile_pool(name="recp", bufs=2) as recp,
            tc.tile_pool(name="orp", bufs=3) as orp,
            tc.tile_pool(name="rp", bufs=2) as rp,
            tc.tile_pool(name="outp", bufs=4) as outp,
            tc.tile_pool(name="ps_a", bufs=2, space="PSUM") as ps_a,
            tc.tile_pool(name="ps_s", bufs=2, space="PSUM") as ps_s,
            tc.tile_pool(name="ps_o", bufs=2, space="PSUM") as ps_o,
        ):
            zero_g = nc.gpsimd.to_reg(0.0)

            # ---- constants / weights (bf16 direct, no staging casts) ------
            bq_sb = consts.tile([128, 4], f32)
            bk_sb = consts.tile([128, 4], f32)

            # persistent activations
            xT = acts.tile([128, 8, S], cdt)       # [d%128, dblk, seq]
            qT = acts.tile([128, 4, S], cdt)       # [col%128, colblk, seq]
            kT = acts.tile([128, 4, S], cdt)
            # partition-swapped copies: head pair (A, B) lives at partitions
            # (64:128, 0:64) instead of (0:64, 64:128). S matmuls alternate
            # halves per key block so each head's next LDWEIGHTS loads into
            # the idle 64-row half while the other half's matmul streams.
            qT2 = acts.tile([128, 4, S], cdt)
            kT2 = acts.tile([128, 4, S], cdt)
            v_ext = acts.tile([128, NKB, HPC * (HD + 1)], cdt)  # per kb: 8*(64+1)
            oT = acts.tile([128, 4, S], cdt)
            for l in range(HPC):                   # ones columns for denominator
                nc.vector.memset(v_ext[:, :, 65 * l + 64: 65 * l + 65], 1.0)

            wq_sb = consts.tile([128, 8, LCOL], cdt)
            wk_sb = consts.tile([128, 8, LCOL], cdt)
            wv_sb = consts.tile([128, 8, LCOL], cdt)
            wp_sb = consts.tile([128, 4, D], cdt)

            def load_weights_early():
                # xt(g0) is already queued ahead of these; wp (only needed by
                # the g0 projection, which runs during g1) loads last, and the
                # tiny biases load after the weights they are added to.
                nc.sync.dma_start(out=wq_sb, in_=wq_d.ap()
                                  .rearrange("(c p) n -> p c n", p=128))
                nc.sync.dma_start(out=wk_sb, in_=wk_d.ap()
                                  .rearrange("(c p) n -> p c n", p=128))
                nc.sync.dma_start(out=bq_sb,
                                  in_=bq_d.ap().rearrange("(c p) -> p c", p=128))
                nc.sync.dma_start(out=bk_sb,
                                  in_=bk_d.ap().rearrange("(c p) -> p c", p=128))
                nc.sync.dma_start(out=wv_sb, in_=wv_d.ap()
                                  .rearrange("(c p) n -> p c n", p=128))

            def phase1_gen(g):
                """qk/v for seq group g; yields between chunks for interleave."""
                nc.sync.dma_start(
                    out=xT[:, :, g * SG:(g + 1) * SG],
                    in_=xt_d.ap()[g].rearrange("(c p) s -> p c s", p=128))
                yield
                for w_sb, b_sb, dstT, dstT2 in ((wq_sb, bq_sb, qT, qT2),
                                                (wk_sb, bk_sb, kT, kT2)):
                    for m in range(4):
                        pq = ps_a.tile([128, SG], f32, name="pq", tag="ps_a")
                        for dc in range(8):
                            nc.tensor.matmul(
                                pq, lhsT=w_sb[:, dc, 128 * m:128 * (m + 1)],
                                rhs=xT[:, dc, g * SG:(g + 1) * SG],
                                start=(dc == 0), stop=(dc == 7))
                        g_sl = slice(g * SG, (g + 1) * SG)
                        nc.vector.tensor_scalar_add(
                            dstT[:, m, g_sl], pq, b_sb[:, m:m + 1])
                        nc.sync.dma_start(out=dstT2[64:128, m, g_sl],
                                          in_=dstT[0:64, m, g_sl])
                        nc.sync.dma_start(out=dstT2[0:64, m, g_sl],
                                          in_=dstT[64:128, m, g_sl])
                        yield
                for s in range(4):
                    pv = ps_a.tile([128, LCOL], f32, name="pv", tag="ps_a")
                    for dc in range(8):
                        nc.tensor.matmul(
                            pv, lhsT=xT[:, dc, g * SG + 128 * s:g * SG + 128 * (s + 1)],
                            rhs=wv_sb[:, dc, :], start=(dc == 0), stop=(dc == 7))
                    kb = 4 * g + s
                    nc.vector.tensor_copy(
                        v_ext[:, kb, :].rearrange("p (h e) -> p h e", e=65)[:, :, 0:64],
                        pv.rearrange("p (h e) -> p h e", e=64))
                    yield

            def attn(hp, qg, tick=lambda: None, queue=None):
                """One (head-pair, q-group) attention unit for heads (2hp,
                2hp+1). The two heads' K=64 score matmuls target different
                64-row PE tiles (partitions 0-63 / 64-127) and run
                concurrently; one exp / affine_select instruction covers both
                heads' blocks for the same key block. exp/AV emission lags S
                emission via `queue` (shared across units); the normalize
                chain is staged so no engine FIFO ever waits on a slow hop."""
                nkb = 4 * qg + 4
                q_sl = slice(qg * SG, (qg + 1) * SG)
                psum_os = [ps_o.tile([65, SG], f32, name=f"psum_o{j}",
                                     tag="ps_o") for j in range(2)]
                st = {}

                def exp_av(kb, psum_s):
                    p_sb = pp.tile([128, 2, SG], cdt, name="p_sb", tag="p_sb")
                    qoff = 128 * kb - 512 * qg
                    if qoff > 0:
                        nc.scalar.activation(p_sb[:, :, qoff:SG],
                                             psum_s[:, :, qoff:SG], Exp,
                                             scale=0.125)
                        nc.vector.memset(p_sb[:, :, 0:qoff], 0.0)
                    else:
                        nc.scalar.activation(p_sb, psum_s, Exp, scale=0.125)
                    if kb >= 4 * qg:  # diagonal block: triangle strip, both j
                        q0 = max(qoff, 0)
                        w = min(128, SG - q0)
                        nc.gpsimd.affine_select(
                            out=p_sb[:, :, q0:q0 + w],
                            in_=p_sb[:, :, q0:q0 + w],
                            compare_op=GE, fill=zero_g,
                            base=512 * qg + q0 - 128 * kb,
                            channel_multiplier=-1, pattern=[[0, 2], [1, w]])
                    for j in range(2):
                        h = 2 * hp + j
                        nc.tensor.matmul(
                            psum_os[j], lhsT=v_ext[:, kb, 65 * h:65 * h + 65],
                            rhs=p_sb[:, j, :], start=(kb == 0),
                            stop=(kb == nkb - 1))

                def stage_a(_a, _b):
                    for j in range(2):
                        o_raw = orp.tile([65, SG], f32, name=f"o_raw{j}",
                                         tag="o_raw")
                        nc.vector.tensor_copy(o_raw, psum_os[j])
                        # den row to physical partition 0 (custom DVE/gpsimd
                        # ops misbehave at non-zero base partitions)
                        den0 = recp.tile([1, SG], f32, name=f"den0{j}",
                                         tag="den0")
                        nc.sync.dma_start(out=den0, in_=o_raw[64:65, :])
                        st[("o_raw", j)], st[("den0", j)] = o_raw, den0

                def stage_b(_a, _b):
                    for j in range(2):
                        rec = recp.tile([1, SG], f32, name=f"rec{j}",
                                        tag="rec")
                        nc.vector.reciprocal_approx_fast(rec, st[("den0", j)])
                        r_sb = rp.tile([64, SG], f32, name=f"r_sb{j}",
                                       tag="r_sb")
                        nc.gpsimd.partition_broadcast(r_sb, rec, channels=64)
                        st[("r_sb", j)] = r_sb

                def stage_c(_a, _b):
                    for j in range(2):
                        po_sl = slice(64 * j, 64 * j + 64)
                        nc.vector.tensor_mul(oT[po_sl, hp, q_sl],
                                             st[("o_raw", j)][0:64, :],
                                             st[("r_sb", j)])

                # Two key blocks of S before their AVs: the 64-row S tiles
                # and full-array AV matmuls force a PE mode switch at each
                # transition, so batch to halve the switch count.
                for kb2 in range(0, nkb, 2):
                    pss = []
                    for kb in (kb2, kb2 + 1):
                        psum_s = ps_s.tile([128, 2, SG], f32, name="psum_s",
                                           tag="ps_s")
                        for j in range(2):
                            # head j's k/q from the original tiles on even
                            # (kb+j), the partition-swapped ones on odd, so
                            # consecutive key blocks alternate PE row halves
                            # and the weight loads overlap the running matmul
                            if kb % 2 == 0:
                                kk, qq = kT, qT
                                po_sl = slice(64 * j, 64 * j + 64)
                            else:
                                kk, qq = kT2, qT2
                                po_sl = slice(64 * (1 - j), 64 * (1 - j) + 64)
                            nc.tensor.matmul(
                                psum_s[:, j, :],
                                lhsT=kk[po_sl, hp, 128 * kb:128 * (kb + 1)],
                                rhs=qq[po_sl, hp, q_sl], start=True, stop=True)
                            tick()
                        pss.append((kb, psum_s))
                    for kb, ps in pss:
                        queue.append((exp_av, kb, ps))
                        if len(queue) > 2:
                            fn, a, b = queue.pop(0)
                            fn(a, b)
                queue.append((stage_a, None, None))
                queue.append((stage_b, None, None))
                queue.append((stage_c, None, None))
                while len(queue) > 4:
                    fn, a, b = queue.pop(0)
                    fn(a, b)

            def proj_gen(g):
                for s in range(4):
                    sb = 4 * g + s
                    for j in range(2):
                        ppr = ps_a.tile([128, SG], f32, name="ppr", tag="ps_a")
                        for c in range(4):
                            nc.tensor.matmul(
                                ppr, lhsT=oT[:, c, 128 * sb:128 * (sb + 1)],
                                rhs=wp_sb[:, c, j * SG:(j + 1) * SG],
                                start=(c == 0), stop=(c == 3))
                        o_sb = outp.tile([128, SG], cdt, name="o_sb", tag="o_sb")
                        nc.vector.tensor_copy(o_sb, ppr)
                        nc.sync.dma_start(
                            out=out_d.ap()[128 * sb:128 * (sb + 1),
                                           j * SG:(j + 1) * SG],
                            in_=o_sb)
                        yield

            # Emission interleaves last-group projections and next-group qkv
            # into the exp-bound attention stretches so the PE stays fed.
            p1_0 = phase1_gen(0)
            next(p1_0)            # queue the xt(g0) DMA first...
            load_weights_early()  # ...then the weights it computes against
            for _ in p1_0:
                pass
            nc.sync.dma_start(out=wp_sb, in_=wp_d.ap()
                              .rearrange("(c p) n -> p c n", p=128))
            for g in range(NSG):
                fill = []
                if g > 0:
                    fill.append(proj_gen(g - 1))
                if g < NSG - 1:
                    fill.append(phase1_gen(g + 1))
                n_chunks = (8 if g > 0 else 0) + (13 if g < NSG - 1 else 0)
                n_ticks = HPC * (4 * g + 4)
                stride = max(1, n_ticks // max(n_chunks, 1))
                state = {"i": 0}

                def feed():
                    if fill:
                        try:
                            next(fill[0])
                        except StopIteration:
                            fill.pop(0)

                def tick():
                    state["i"] += 1
                    if state["i"] % stride == 0:
                        feed()

                queue = []
                for hp in range(HPC // 2):
                    attn(hp, g, tick, queue)
                while queue:  # group boundary: drain before proj fill reads oT
                    fn, a, b = queue.pop(0)
                    fn(a, b)
                    feed()  # keep the PE fed during the drain
                for gen in fill:  # drain any remaining chunks
                    for _ in gen:
                        pass
            for _ in proj_gen(NSG - 1):
                pass

            if debug_dump:
                for nm, t in (("qT", qT), ("kT", kT), ("v_ext", v_ext),
                              ("oT", oT)):
                    dmp = nc.dram_tensor(f"dump_{nm}", list(t.shape), cdt,
                                         kind="ExternalOutput")
                    nc.sync.dma_start(out=dmp.ap(), in_=t)

    nc.compile()
    return nc


def _get_nc():
    if "nc" not in _CACHE:
        _CACHE["nc"] = _build()
    return _CACHE["nc"]


def make_in_maps(x, Wq, bq, Wk, bk, Wv, Wp):
    import ml_dtypes
    bf16 = ml_dtypes.bfloat16
    in_maps = []
    for c in range(8):
        b, hg = c // 2, c % 2
        hs = slice(hg * HPC, (hg + 1) * HPC)
        in_maps.append({
            "xt": np.ascontiguousarray(
                x[b].T.reshape(D, NSG, SG).transpose(1, 0, 2).astype(bf16)),
            "wq": np.ascontiguousarray(
                Wq[hs].transpose(1, 0, 2).reshape(D, LCOL).astype(bf16)),
            "wk": np.ascontiguousarray(
                Wk[hs].transpose(1, 0, 2).reshape(D, LCOL).astype(bf16)),
            "wv": np.ascontiguousarray(
                Wv[hs].transpose(1, 0, 2).reshape(D, LCOL).astype(bf16)),
            "bq": np.ascontiguousarray(bq[hs].reshape(LCOL).astype(np.float32)),
            "bk": np.ascontiguousarray(bk[hs].reshape(LCOL).astype(np.float32)),
            "wp": np.ascontiguousarray(
                Wp[hg * LCOL:(hg + 1) * LCOL, :].astype(bf16)),
        })
    return in_maps


def combine(results, Wp, bv, bp):
    """Unshard: sum the two head-group partials per batch + linear bias terms."""
    add = bp + bv.reshape(D) @ Wp
    out = np.empty((B, S, D), np.float32)
    for b in range(B):
        out[b] = (results[2 * b]["out"].astype(np.float32)
                  + results[2 * b + 1]["out"].astype(np.float32) + add)
    return out


def kernel(x, Wq, bq, Wk, bk, Wv, bv, Wp, bp):
    from concourse.bass_utils import run_bass_kernel_spmd

    x = np.asarray(x, np.float32)
    Wq = np.asarray(Wq, np.float32)
    Wk = np.asarray(Wk, np.float32)
    Wv = np.asarray(Wv, np.float32)
    bq = np.asarray(bq, np.float32)
    bk = np.asarray(bk, np.float32)
    bv = np.asarray(bv, np.float32)
    Wp = np.asarray(Wp, np.float32)
    bp = np.asarray(bp, np.float32)

    nc = _get_nc()
    in_maps = make_in_maps(x, Wq, bq, Wk, bk, Wv, Wp)
    res = run_bass_kernel_spmd(nc, in_maps, core_ids=list(range(8)))
    return combine(res.results, Wp, bv, bp)


# revision 31
# speedup vs baseline: 1.0022x; 1.0016x over previous
"""Multi-head causal attention (B=4, S=2048, D=1024, H=16) on 8 TRN2 cores.

Sharding: data-parallel over batch (4) x tensor-parallel over heads (2 groups
of 8 heads). Core c handles batch c//2, head-group c%2. Each core computes
q/k/v projections for its 8 heads, causal flash-style attention, and a partial
output projection against its row-shard of Wp. Host sums the two partials per
batch and adds the bias terms (bp + bv @ Wp, which commute with the row-sum).

v2 pipeline notes:
- Host passes x pre-transposed ([D, S]) and all big operands in bf16: no
  on-chip casts, no PE transposes, and half the startup DMA bytes.
- Causal masking via gpsimd.affine_select (triangle strips) + DVE memsets for
  the fully-masked column ranges; no mask tensor at all. This keeps the mask
  work off the Vector FIFO, which previously head-of-line-blocked the AV
  matmuls behind the normalize chain and let the PE HAM re-throttle.
- The softmax-denominator reciprocal still round-trips through DRAM to spread
  512 lanes, but is split into stages (A: evacuate+gather DMAs, B: reciprocal+
  scatter DMAs, C: normalize multiply) that pop from the shared pipeline queue
  one-per-S-matmul, so each stage's DMA completes before its consumer issues.
"""

import numpy as np

B, S, D, H = 4, 2048, 1024, 16
HD = D // H            # 64
HPC = 8                # heads per core
LCOL = HPC * HD        # 512 local columns
NSG = 4                # seq groups of 512
SG = S // NSG          # 512
NKB = S // 128         # 16 key blocks of 128

_CACHE = {}


def _build(cdt_name="bfloat16", debug_dump=False):
    import concourse.bass as bass
    import concourse.tile as tile
    from concourse import bacc, mybir

    f32 = mybir.dt.float32
    cdt = getattr(mybir.dt, cdt_name)

    nc = bacc.Bacc("TRN2", target_bir_lowering=False, debug=False)

    xt_d = nc.dram_tensor("xt", [NSG, D, SG], cdt, kind="ExternalInput")
    wq_d = nc.dram_tensor("wq", [D, LCOL], cdt, kind="ExternalInput")
    wk_d = nc.dram_tensor("wk", [D, LCOL], cdt, kind="ExternalInput")
    wv_d = nc.dram_tensor("wv", [D, LCOL], cdt, kind="ExternalInput")
    bq_d = nc.dram_tensor("bq", [LCOL], f32, kind="ExternalInput")
    bk_d = nc.dram_tensor("bk", [LCOL], f32, kind="ExternalInput")
    wp_d = nc.dram_tensor("wp", [LCOL, D], cdt, kind="ExternalInput")
    out_d = nc.dram_tensor("out", [S, D], cdt, kind="ExternalOutput")

    Exp = mybir.ActivationFunctionType.Exp
    GE = mybir.AluOpType.is_ge

    with tile.TileContext(nc) as tc:
        with (
            tc.tile_pool(name="consts", bufs=1) as consts,
            tc.tile_pool(name="acts", bufs=1) as acts,
            tc.tile_pool(name="pp", bufs=6) as pp,
            tc.tile_pool(name="recp", bufs=2) as recp,
            tc.tile_pool(name="orp", bufs=3) as orp,
            tc.tile_pool(name="rp", bufs=2) as rp,
            tc.tile_pool(name="outp", bufs=4) as outp,
            tc.tile_pool(name="ps_a", bufs=2, space="PSUM") as ps_a,
            tc.tile_pool(name="ps_s", bufs=2, space="PSUM") as ps_s,
            tc.tile_pool(name="ps_o", bufs=2, space="PSUM") as ps_o,
        ):
            zero_g = nc.gpsimd.to_reg(0.0)

            # ---- constants / weights (bf16 direct, no staging casts) ------
            bq_sb = consts.tile([128, 4], f32)
            bk_sb = consts.tile([128, 4], f32)

            # persistent activations
            xT = acts.tile([128, 8, S], cdt)       # [d%128, dblk, seq]
            qT = acts.tile([128, 4, S], cdt)       # [col%128, colblk, seq]
            kT = acts.tile([128, 4, S], cdt)
            v_ext = acts.tile([128, NKB, HPC * (HD + 1)], cdt)  # per kb: 8*(64+1)
            oT = acts.tile([128, 4, S], cdt)
            for l in range(HPC):                   # ones columns for denominator
                nc.vector.memset(v_ext[:, :, 65 * l + 64: 65 * l + 65], 1.0)

            wq_sb = consts.tile([128, 8, LCOL], cdt)
            wk_sb = consts.tile([128, 8, LCOL], cdt)
            wv_sb = consts.tile([128, 8, LCOL], cdt)
            wp_sb = consts.tile([128, 4, D], cdt)

            def load_weights_early():
                # xt(g0) is already queued ahead of these; wp (only needed by
                # the g0 projection, which runs during g1) loads last, and the
                # tiny biases load after the weights they are added to.
                nc.sync.dma_start(out=wq_sb, in_=wq_d.ap()
                                  .rearrange("(c p) n -> p c n", p=128))
                nc.sync.dma_start(out=wk_sb, in_=wk_d.ap()
                                  .rearrange("(c p) n -> p c n", p=128))
                nc.sync.dma_start(out=bq_sb,
                                  in_=bq_d.ap().rearrange("(c p) -> p c", p=128))
                nc.sync.dma_start(out=bk_sb,
                                  in_=bk_d.ap().rearrange("(c p) -> p c", p=128))
                nc.sync.dma_start(out=wv_sb, in_=wv_d.ap()
                                  .rearrange("(c p) n -> p c n", p=128))

            def phase1_gen(g):
                """qk/v for seq group g; yields between chunks for interleave."""
                nc.sync.dma_start(
                    out=xT[:, :, g * SG:(g + 1) * SG],
                    in_=xt_d.ap()[g].rearrange("(c p) s -> p c s", p=128))
                yield
                for w_sb, b_sb, dstT in ((wq_sb, bq_sb, qT), (wk_sb, bk_sb, kT)):
                    for m in range(4):
                        pq = ps_a.tile([128, SG], f32, name="pq", tag="ps_a")
                        for dc in range(8):
                            nc.tensor.matmul(
                                pq, lhsT=w_sb[:, dc, 128 * m:128 * (m + 1)],
                                rhs=xT[:, dc, g * SG:(g + 1) * SG],
                                start=(dc == 0), stop=(dc == 7))
                        nc.vector.tensor_scalar_add(
                            dstT[:, m, g * SG:(g + 1) * SG], pq,
                            b_sb[:, m:m + 1])
                        yield
                for s in range(4):
                    pv = ps_a.tile([128, LCOL], f32, name="pv", tag="ps_a")
                    for dc in range(8):
                        nc.tensor.matmul(
                            pv, lhsT=xT[:, dc, g * SG + 128 * s:g * SG + 128 * (s + 1)],
                            rhs=wv_sb[:, dc, :], start=(dc == 0), stop=(dc == 7))
                    kb = 4 * g + s
                    nc.vector.tensor_copy(
                        v_ext[:, kb, :].rearrange("p (h e) -> p h e", e=65)[:, :, 0:64],
                        pv.rearrange("p (h e) -> p h e", e=64))
                    yield

            def attn(hp, qg, tick=lambda: None, queue=None):
                """One (head-pair, q-group) attention unit for heads (2hp,
                2hp+1). The two heads' K=64 score matmuls target different
                64-row PE tiles (partitions 0-63 / 64-127) and run
                concurrently; one exp / affine_select instruction covers both
                heads' blocks for the same key block. exp/AV emission lags S
                emission via `queue` (shared across units); the normalize
                chain is staged so no engine FIFO ever waits on a slow hop."""
                nkb = 4 * qg + 4
                q_sl = slice(qg * SG, (qg + 1) * SG)
                psum_os = [ps_o.tile([65, SG], f32, name=f"psum_o{j}",
                                     tag="ps_o") for j in range(2)]
                st = {}

                def exp_av(kb, psum_s):
                    p_sb = pp.tile([128, 2, SG], cdt, name="p_sb", tag="p_sb")
                    qoff = 128 * kb - 512 * qg
                    if qoff > 0:
                        nc.scalar.activation(p_sb[:, :, qoff:SG],
                                             psum_s[:, :, qoff:SG], Exp,
                                             scale=0.125)
                        nc.vector.memset(p_sb[:, :, 0:qoff], 0.0)
                    else:
                        nc.scalar.activation(p_sb, psum_s, Exp, scale=0.125)
                    if kb >= 4 * qg:  # diagonal block: triangle strip, both j
                        q0 = max(qoff, 0)
                        w = min(128, SG - q0)
                        nc.gpsimd.affine_select(
                            out=p_sb[:, :, q0:q0 + w],
                            in_=p_sb[:, :, q0:q0 + w],
                            compare_op=GE, fill=zero_g,
                            base=512 * qg + q0 - 128 * kb,
                            channel_multiplier=-1, pattern=[[0, 2], [1, w]])
                    for j in range(2):
                        h = 2 * hp + j
                        nc.tensor.matmul(
                            psum_os[j], lhsT=v_ext[:, kb, 65 * h:65 * h + 65],
                            rhs=p_sb[:, j, :], start=(kb == 0),
                            stop=(kb == nkb - 1))

                def stage_a(_a, _b):
                    for j in range(2):
                        o_raw = orp.tile([65, SG], f32, name=f"o_raw{j}",
                                         tag="o_raw")
                        nc.vector.tensor_copy(o_raw, psum_os[j])
                        # den row to physical partition 0 (custom DVE/gpsimd
                        # ops misbehave at non-zero base partitions)
                        den0 = recp.tile([1, SG], f32, name=f"den0{j}",
                                         tag="den0")
                        nc.sync.dma_start(out=den0, in_=o_raw[64:65, :])
                        st[("o_raw", j)], st[("den0", j)] = o_raw, den0

                def stage_b(_a, _b):
                    for j in range(2):
                        rec = recp.tile([1, SG], f32, name=f"rec{j}",
                                        tag="rec")
                        nc.vector.reciprocal_approx_fast(rec, st[("den0", j)])
                        r_sb = rp.tile([64, SG], f32, name=f"r_sb{j}",
                                       tag="r_sb")
                        nc.gpsimd.partition_broadcast(r_sb, rec, channels=64)
                        st[("r_sb", j)] = r_sb

                def stage_c(_a, _b):
                    for j in range(2):
                        po_sl = slice(64 * j, 64 * j + 64)
                        nc.vector.tensor_mul(oT[po_sl, hp, q_sl],
                                             st[("o_raw", j)][0:64, :],
                                             st[("r_sb", j)])

                # Two key blocks of S before their AVs: the 64-row S tiles
                # and full-array AV matmuls force a PE mode switch at each
                # transition, so batch to halve the switch count.
                for kb2 in range(0, nkb, 2):
                    pss = []
                    for kb in (kb2, kb2 + 1):
                        psum_s = ps_s.tile([128, 2, SG], f32, name="psum_s",
                                           tag="ps_s")
                        for j in range(2):
                            po_sl = slice(64 * j, 64 * j + 64)
                            nc.tensor.matmul(
                                psum_s[:, j, :],
                                lhsT=kT[po_sl, hp, 128 * kb:128 * (kb + 1)],
                                rhs=qT[po_sl, hp, q_sl], start=True, stop=True)
                            tick()
                        pss.append((kb, psum_s))
                    for kb, ps in pss:
                        queue.append((exp_av, kb, ps))
                        if len(queue) > 2:
                            fn, a, b = queue.pop(0)
                            fn(a, b)
                queue.append((stage_a, None, None))
                queue.append((stage_b, None, None))
                queue.append((stage_c, None, None))
                while len(queue) > 4:
                    fn, a, b = queue.pop(0)
                    fn(a, b)

            def proj_gen(g):
                for s in range(4):
                    sb = 4 * g + s
                    for j in range(2):
                        ppr = ps_a.tile([128, SG], f32, name="ppr", tag="ps_a")
                        for c in range(4):
                            nc.tensor.matmul(
                                ppr, lhsT=oT[:, c, 128 * sb:128 * (sb + 1)],
                                rhs=wp_sb[:, c, j * SG:(j + 1) * SG],
                                start=(c == 0), stop=(c == 3))
                        o_sb = outp.tile([128, SG], cdt, name="o_sb", tag="o_sb")
                        nc.vector.tensor_copy(o_sb, ppr)
                        nc.sync.dma_start(
                            out=out_d.ap()[128 * sb:128 * (sb + 1),
                                           j * SG:(j + 1) * SG],
                            in_=o_sb)
                        yield

            # Emission interleaves last-group projections and next-group qkv
            # into the exp-bound attention stretches so the PE stays fed.
            p1_0 = phase1_gen(0)
            next(p1_0)            # queue the xt(g0) DMA first...
            load_weights_early()  # ...then the weights it computes against
            for _ in p1_0:
                pass
            nc.sync.dma_start(out=wp_sb, in_=wp_d.ap()
                              .rearrange("(c p) n -> p c n", p=128))
            for g in range(NSG):
                fill = []
                if g > 0:
                    fill.append(proj_gen(g - 1))
                if g < NSG - 1:
                    fill.append(phase1_gen(g + 1))
                n_chunks = (8 if g > 0 else 0) + (13 if g < NSG - 1 else 0)
                n_ticks = HPC * (4 * g + 4)
                stride = max(1, n_ticks // max(n_chunks, 1))
                state = {"i": 0}

                def feed():
                    if fill:
                        try:
                            next(fill[0])
                        except StopIteration:
                            fill.pop(0)

                def tick():
                    state["i"] += 1
                    if state["i"] % stride == 0:
                        feed()

                queue = []
                for hp in range(HPC // 2):
                    attn(hp, g, tick, queue)
                while queue:  # group boundary: drain before proj fill reads oT
                    fn, a, b = queue.pop(0)
                    fn(a, b)
                    feed()  # keep the PE fed during the drain
                for gen in fill:  # drain any remaining chunks
                    for _ in gen:
                        pass
            for _ in proj_gen(NSG - 1):
                pass

            if debug_dump:
                for nm, t in (("qT", qT), ("kT", kT), ("v_ext", v_ext),
                              ("oT", oT)):
                    dmp = nc.dram_tensor(f"dump_{nm}", list(t.shape), cdt,
                                         kind="ExternalOutput")
                    nc.sync.dma_start(out=dmp.ap(), in_=t)

    nc.compile()
    return nc


def _get_nc():
    if "nc" not in _CACHE:
        _CACHE["nc"] = _build()
    return _CACHE["nc"]


def make_in_maps(x, Wq, bq, Wk, bk, Wv, Wp):
    import ml_dtypes
    bf16 = ml_dtypes.bfloat16
    in_maps = []
    for c in range(8):
        b, hg = c // 2, c % 2
        hs = slice(hg * HPC, (hg + 1) * HPC)
        in_maps.append({
            "xt": np.ascontiguousarray(
                x[b].T.reshape(D, NSG, SG).transpose(1, 0, 2).astype(bf16)),
            "wq": np.ascontiguousarray(
                Wq[hs].transpose(1, 0, 2).reshape(D, LCOL).astype(bf16)),
            "wk": np.ascontiguousarray(
                Wk[hs].transpose(1, 0, 2).reshape(D, LCOL).astype(bf16)),
            "wv": np.ascontiguousarray(
                Wv[hs].transpose(1, 0, 2).reshape(D, LCOL).astype(bf16)),
            "bq": np.ascontiguousarray(bq[hs].reshape(LCOL).astype(np.float32)),
            "bk": np.ascontiguousarray(bk[hs].reshape(LCOL).astype(np.float32)),
            "wp": np.ascontiguousarray(
                Wp[hg * LCOL:(hg + 1) * LCOL, :].astype(bf16)),
        })
    return in_maps


def combine(results, Wp, bv, bp):
    """Unshard: sum the two head-group partials per batch + linear bias terms."""
    add = bp + bv.reshape(D) @ Wp
    out = np.empty((B, S, D), np.float32)
    for b in range(B):
        out[b] = (results[2 * b]["out"].astype(np.float32)
                  + results[2 * b + 1]["out"].astype(np.float32) + add)
    return out


def kernel(x, Wq, bq, Wk, bk, Wv, bv, Wp, bp):
    from concourse.bass_utils import run_bass_kernel_spmd

    x = np.asarray(x, np.float32)
    Wq = np.asarray(Wq, np.float32)
    Wk = np.asarray(Wk, np.float32)
    Wv = np.asarray(Wv, np.float32)
    bq = np.asarray(bq, np.float32)
    bk = np.asarray(bk, np.float32)
    bv = np.asarray(bv, np.float32)
    Wp = np.asarray(Wp, np.float32)
    bp = np.asarray(bp, np.float32)

    nc = _get_nc()
    in_maps = make_in_maps(x, Wq, bq, Wk, bk, Wv, Wp)
    res = run_bass_kernel_spmd(nc, in_maps, core_ids=list(range(8)))
    return combine(res.results, Wp, bv, bp)
